# revision 5
# baseline (speedup 1.0000x reference)
"""CrissCrossAttention TRN2 kernel v4 — fp32r datapath, software-pipelined.

Math: softmax row-constants cancel, so attn = softmax_j(y_i . x_j) with
y = (Wq^T Wk)^T x + Wk^T bq; gamma folded into Wv/bv on the host.

Per core (B=1 image), two passes: column attention writes gamma*h_out to a
DRAM scratch hT[c, w, j] (contiguous stores); row attention computes
gamma*w_out, adds x + h' (GPSIMD) and stores final rows.

Matmuls stream FP32R (e8m11 — fp32 range, 11-bit mantissa, full PE rate at
N>=256; ~2.4e-4 operand rounding). All matmul operands are produced as f32r:
x / G / Wv rounded on the host and DMA'd as f32r; y via DVE bias-add cast;
exp output cast by ACT. Matmul outputs accumulate in full fp32 PSUM.

Pipeline: quads of 4 slices. produce = vT projections (PSUM evacuated at once
to SBUF), QK logits, exps (slices 0-1 as one wide [128,512] exp + batched DVE
rowsum; slices 2-3 as [128,256] exps with ACT accum rowsums). consume runs per
half-quad (reciprocal, in-place vT scale, AV matmuls) lagging DEPTH half-quads
so sequencer waits are pre-satisfied. The next block's x load + y projection
chunks are interleaved between quads.
"""

import sys

import numpy as np

for _p in ("/opt/trn_rl_repo",):
    if _p not in sys.path:
        sys.path.insert(0, _p)

from contextlib import ExitStack

import concourse.bacc as bacc
import concourse.bass as bass
import concourse.mybir as mybir
import concourse.tile as tile
from concourse import bass_utils

F32 = mybir.dt.float32
F32R = mybir.dt.float32r
EXP = mybir.ActivationFunctionType.Exp

C = 64


def _f(ap):
    return ap.bitcast(F32)


def round_f32r(a):
    """Round fp32 array to fp32r (e8m11) bits, round-to-nearest."""
    u = np.ascontiguousarray(a, np.float32).view(np.uint32).copy()
    u = (u + np.uint32(0x800)) & np.uint32(0xFFFFF000)
    return u.view(np.float32)


def build_program(H, W, TW, TH, Gr_np, Wvb_np, r_np):
    nc = bacc.Bacc(
        "TRN2", target_bir_lowering=False, debug=False, enable_asserts=False
    )
    assert H == 256 and W == 256 and TW % 4 == 0 and TH % 4 == 0
    x_d = nc.dram_tensor("x", [C, H, W], F32, kind="ExternalInput")
    out_d = nc.dram_tensor("out", [C, H, W], F32, kind="ExternalOutput")
    Gr_d = nc.inline_tensor(np.ascontiguousarray(round_f32r(Gr_np)), name="Gr")
    Wvb_d = nc.inline_tensor(np.ascontiguousarray(round_f32r(Wvb_np)), name="Wvb")
    r64_d = nc.inline_tensor(r_np.astype(np.float32).reshape(C, 1), name="r64")
    ones_d = nc.inline_tensor(np.ones((1, 512), np.float32), name="ones1")

    with ExitStack() as ctx:
        tc = ctx.enter_context(tile.TileContext(nc))
        _body(ctx, tc, nc, x_d.ap(), out_d.ap(), Gr_d.ap(), Wvb_d.ap(),
              r64_d.ap(), ones_d.ap(), H, W, TW, TH)
    nc.compile()
    return nc


class _Quad:
    __slots__ = ("vtsg", "eg", "ssumg", "L", "store", "avq")

    def __init__(self, vtsg, eg, ssumg, L, store):
        self.vtsg = vtsg
        self.eg = eg
        self.ssumg = ssumg
        self.L = L
        self.store = store
        self.avq = None


def _body(ctx, tc, nc, x, out, Gr, Wvb, r64, ones, H, W, TW, TH):
    consts = ctx.enter_context(tc.tile_pool(name="consts", bufs=1))
    blocks = ctx.enter_context(tc.tile_pool(name="blocks", bufs=2))
    work = ctx.enter_context(tc.tile_pool(name="work", bufs=2))
    epool = ctx.enter_context(tc.tile_pool(name="epool", bufs=4))
    spool = ctx.enter_context(tc.tile_pool(name="spool", bufs=4))
    psum_av = ctx.enter_context(tc.tile_pool(name="psum_av", bufs=1, space="PSUM"))
    psum_lp = ctx.enter_context(tc.tile_pool(name="psum_lp", bufs=4, space="PSUM"))
    psum_vt = ctx.enter_context(tc.tile_pool(name="psum_vt", bufs=1, space="PSUM"))
    psum_yp = ctx.enter_context(tc.tile_pool(name="psum_yp", bufs=1, space="PSUM"))
    dram = ctx.enter_context(tc.tile_pool(name="dram", bufs=1, space="DRAM"))

    Gr_sb = consts.tile([C + 1, C], F32R)
    nc.sync.dma_start(out=Gr_sb[:], in_=Gr.bitcast(F32R))
    Wvb_sb = consts.tile([C + 1, C], F32R)
    nc.sync.dma_start(out=Wvb_sb[:], in_=Wvb.bitcast(F32R))
    r_sb = consts.tile([C, 1], F32)
    nc.sync.dma_start(out=r_sb[:], in_=r64)

    hT = dram.tile([C, W, H], F32)  # gamma*h_out as [c, w, j]
    HT_ap = hT[:]

    # ---------------- software pipeline over half-quads ----------------
    pending = []
    DEPTH = 5

    def produce_quad(lhsT_x, rhs_x, lhsT_y, L, store_quad):
        NI = L // 128
        vtpg = psum_vt.tile([128, 4, NI, C], F32, tag="vtpg")
        eg = epool.tile([128, 4, NI, L], F32R, tag="eg")
        ssumg = spool.tile([128, 4 * NI], F32, tag="ssumg")
        for s in range(4):
            for i in range(NI):
                nc.tensor.matmul(
                    vtpg[:, s, i, :], lhsT=lhsT_x(s, i), rhs=Wvb_sb[:],
                    start=True, stop=True,
                )
        vtsg = spool.tile([128, 4 * NI, C], F32R, tag="vtsg")
        nc.vector.tensor_copy(vtsg[:], vtpg[:].rearrange("p a b c -> p (a b) c"))
        for s in range(4):
            lp = psum_lp.tile([128, NI, L], F32, tag="lp")
            for i in range(NI):
                nc.tensor.matmul(
                    lp[:, i, :], lhsT=lhsT_y(s, i), rhs=rhs_x(s, i),
                    start=True, stop=True,
                )
                if s >= 2:
                    nc.scalar.activation(
                        eg[:, s, i, :], lp[:, i, :], EXP,
                        accum_out=ssumg[:, s * NI + i : s * NI + i + 1],
                    )
            if s < 2:
                nc.scalar.activation(
                    eg[:, s, :, :].rearrange("p a b -> p (a b)"),
                    lp[:].rearrange("p a b -> p (a b)"),
                    EXP,
                )
            if s == 1:
                nc.vector.reduce_sum(
                    out=ssumg[:, 0 : 2 * NI],
                    in_=_f(eg[:, 0:2, :, :]).rearrange("p a b c -> p (a b) c"),
                    axis=mybir.AxisListType.X,
                )
                if len(pending) > DEPTH:
                    consume_half()
        q = _Quad(vtsg, eg, ssumg, L, store_quad)
        pending.append((q, 0))
        pending.append((q, 1))

    def consume_half():
        q, h = pending.pop(0)
        NI = q.L // 128
        k = 2 * NI
        recg = work.tile([128, k], F32, tag="recg")
        nc.vector.reciprocal(recg[:], q.ssumg[:, h * k : (h + 1) * k])
        nc.vector.tensor_mul(
            q.vtsg[:, h * k : (h + 1) * k, :],
            q.vtsg[:, h * k : (h + 1) * k, :],
            recg[:].broadcast_to([128, k, C]),
        )
        if h == 0:
            q.avq = psum_av.tile([C, 4, q.L], F32, tag="avq")  # 2 banks
        for s in (2 * h, 2 * h + 1):
            for i in range(NI):
                nc.tensor.matmul(
                    q.avq[:, s, :],
                    lhsT=q.vtsg[:, s * NI + i, :],
                    rhs=q.eg[:, s, i, :],
                    start=(i == 0), stop=(i == NI - 1),
                )
        if h == 1:
            q.store(q.avq)

    def step_pipeline():
        while len(pending) > DEPTH:
            consume_half()

    def flush_pipeline():
        while pending:
            consume_half()

    # ---------------- block prologue: x load + y projection ----------------
    def prologue_start_col(wb):
        xaug = blocks.tile([C + 1, H, TW], F32R, tag="xaug")
        nc.sync.dma_start(out=xaug[0:C, :, :], in_=x[:, :, _ts(wb, TW)].bitcast(F32R))
        ones_src = bass.AP(
            tensor=ones.tensor, offset=ones.offset, ap=[[0, 1], [0, H], [1, TW]]
        ).bitcast(F32R)
        nc.gpsimd.dma_start(out=xaug[C : C + 1, :, :], in_=ones_src)
        ypk = blocks.tile([C, H, TW], F32R, tag="ypk")
        return xaug, ypk

    def prologue_start_row(hb):
        xaug = blocks.tile([C + 1, TH, W], F32R, tag="xaug")
        nc.sync.dma_start(out=xaug[0:C, :, :], in_=x[:, _ts(hb, TH), :].bitcast(F32R))
        ones_src = bass.AP(
            tensor=ones.tensor, offset=ones.offset, ap=[[0, 1], [0, TH], [1, W]]
        ).bitcast(F32R)
        nc.gpsimd.dma_start(out=xaug[C : C + 1, :, :], in_=ones_src)
        hbt = blocks.tile([C, W, TH], F32, tag="hb")
        nc.sync.dma_start(out=hbt[:], in_=HT_ap[:, :, _ts(hb, TH)])
        ypk = blocks.tile([C, TH, W], F32R, tag="ypk")
        return xaug, hbt, ypk

    def y_chunk(xaug, ypk, g):
        """One 512-position y chunk: y = G^T x + r (fp32r out)."""
        xaug_f = xaug[:].rearrange("p a b -> p (a b)")
        ypk_f = ypk[:].rearrange("p a b -> p (a b)")
        yp = psum_yp.tile([C, 512], F32, tag="yp")
        nc.tensor.matmul(
            yp[:], lhsT=Gr_sb[:], rhs=xaug_f[:, _ts(g, 512)],
            start=True, stop=True,
        )
        nc.vector.tensor_copy(ypk_f[:, _ts(g, 512)], yp[:])

    # ================= Pass 1: column attention =================
    NBW = W // TW
    NQ = TW // 4
    NCH = H * TW // 512
    CPQ = max(1, NCH // NQ)  # y chunks to emit per quad
    cur = prologue_start_col(0)
    for g in range(NCH):
        y_chunk(cur[0], cur[1], g)
    for wb in range(NBW):
        xaug, ypk = cur
        ypk3 = ypk[:]
        nxt = prologue_start_col(wb + 1) if wb + 1 < NBW else None
        gq = iter(range(NCH))
        for wq in range(NQ):
            wp = wq * 4
            w0 = wb * TW + wp

            def store_quad_col(avq, w0=w0):
                hq = work.tile([C, 4, H], F32, tag="hq")
                nc.scalar.copy(
                    hq[:].rearrange("p a b -> p (a b)"),
                    avq[:].rearrange("p a b -> p (a b)"),
                )
                nc.sync.dma_start(out=HT_ap[:, w0 : w0 + 4, :], in_=hq[:])

            produce_quad(
                lhsT_x=lambda s, i, xaug=xaug, wp=wp: xaug[0 : C + 1, _ts(i, 128), wp + s],
                rhs_x=lambda s, i, xaug=xaug, wp=wp: xaug[0:C, :, wp + s],
                lhsT_y=lambda s, i, ypk3=ypk3, wp=wp: ypk3[:, _ts(i, 128), wp + s],
                L=H,
                store_quad=store_quad_col,
            )
            if nxt is not None:
                for _ in range(CPQ):
                    for g in gq:
                        y_chunk(nxt[0], nxt[1], g)
                        break
            step_pipeline()
        if nxt is not None:
            for g in gq:
                y_chunk(nxt[0], nxt[1], g)
        cur = nxt
    flush_pipeline()

    # ================= Pass 2: row attention + combine =================
    NBH = H // TH
    NQH = TH // 4
    NCHR = TH * W // 512
    CPQR = max(1, NCHR // NQH)
    cur = prologue_start_row(0)
    for g in range(NCHR):
        y_chunk(cur[0], cur[2], g)
    for hb in range(NBH):
        xaug2, hbt, ypk2 = cur
        ypk23 = ypk2[:]
        nxt = prologue_start_row(hb + 1) if hb + 1 < NBH else None
        gq = iter(range(NCHR))
        for hq4 in range(NQH):
            hp = hq4 * 4
            h0 = hb * TH + hp

            xhq = spool.tile([C, 4, W], F32, tag="xhq")
            for s in range(4):
                nc.gpsimd.tensor_add(
                    xhq[:, s, :],
                    _f(xaug2[0:C, hp + s, :]),
                    hbt[:, :, hp + s],
                )

            def store_quad_row(avq, h0=h0, xhq=xhq):
                oq = work.tile([C, 4, W], F32, tag="oq")
                nc.vector.tensor_add(
                    oq[:].rearrange("p a b -> p (a b)"),
                    avq[:].rearrange("p a b -> p (a b)"),
                    xhq[:].rearrange("p a b -> p (a b)"),
                )
                nc.sync.dma_start(out=out[:, h0 : h0 + 4, :], in_=oq[:])

            produce_quad(
                lhsT_x=lambda s, i, xaug2=xaug2, hp=hp: xaug2[0 : C + 1, hp + s, _ts(i, 128)],
                rhs_x=lambda s, i, xaug2=xaug2, hp=hp: xaug2[0:C, hp + s, :],
                lhsT_y=lambda s, i, ypk23=ypk23, hp=hp: ypk23[:, hp + s, _ts(i, 128)],
                L=W,
                store_quad=store_quad_row,
            )
            if nxt is not None:
                for _ in range(CPQR):
                    for g in gq:
                        y_chunk(nxt[0], nxt[2], g)
                        break
            step_pipeline()
        if nxt is not None:
            for g in gq:
                y_chunk(nxt[0], nxt[2], g)
        cur = nxt
    flush_pipeline()


def _ts(i, n):
    return slice(i * n, (i + 1) * n)


def _host_weights(Wq, bq, Wk, bk, Wv, bv, gamma):
    g = float(np.asarray(gamma).reshape(-1)[0])
    G = (Wq.astype(np.float64).T @ Wk.astype(np.float64)).astype(np.float32)
    r = (bq.astype(np.float64) @ Wk.astype(np.float64)).astype(np.float32)
    Gr = np.concatenate([G, r[None, :]], axis=0)
    WvTg = (g * Wv.astype(np.float64).T).astype(np.float32)
    bvg = (g * bv.astype(np.float64)).astype(np.float32)
    Wvb = np.concatenate([WvTg, bvg[None, :]], axis=0)
    return Gr, Wvb, r


LAST_EXEC_NS = None
LAST_RESULT = None


def kernel(x, Wq, bq, Wk, bk, Wv, bv, gamma, _tw=16, _th=16, _trace=False, _tmpdir=None):
    global LAST_EXEC_NS, LAST_RESULT
    x = np.asarray(x, dtype=np.float32)
    B, Cin, H, W = x.shape
    assert Cin == C
    Gr, Wvb, r = _host_weights(
        np.asarray(Wq, np.float32), np.asarray(bq, np.float32),
        np.asarray(Wk, np.float32), np.asarray(bk, np.float32),
        np.asarray(Wv, np.float32), np.asarray(bv, np.float32),
        np.asarray(gamma, np.float32),
    )
    nc = build_program(H, W, _tw, _th, Gr, Wvb, r)
    xr = round_f32r(x)
    in_maps = [{"x": np.ascontiguousarray(xr[b])} for b in range(B)]
    res = bass_utils.run_bass_kernel_spmd(
        nc, in_maps, core_ids=list(range(B)), trace=_trace, tmpdir=_tmpdir
    )
    LAST_RESULT = res
    LAST_EXEC_NS = res.exec_time_ns
    out = np.stack([res.results[b]["out"] for b in range(B)], axis=0)
    return out.astype(np.float32)


# revision 6
# speedup vs baseline: 1.0413x; 1.0413x over previous
"""CrissCrossAttention TRN2 kernel v4 — fp32r datapath, software-pipelined.

Math: softmax row-constants cancel, so attn = softmax_j(y_i . x_j) with
y = (Wq^T Wk)^T x + Wk^T bq; gamma folded into Wv/bv on the host.

Per core (B=1 image), two passes: column attention writes gamma*h_out to a
DRAM scratch hT[c, w, j] (contiguous stores); row attention computes
gamma*w_out, adds x + h' (GPSIMD) and stores final rows.

Matmuls stream FP32R (e8m11 — fp32 range, 11-bit mantissa, full PE rate at
N>=256; ~2.4e-4 operand rounding). All matmul operands are produced as f32r:
x / G / Wv rounded on the host and DMA'd as f32r; y via DVE bias-add cast;
exp output cast by ACT. Matmul outputs accumulate in full fp32 PSUM.

Pipeline: quads of 4 slices. produce = vT projections (PSUM evacuated at once
to SBUF), QK logits, exps (slices 0-1 as one wide [128,512] exp + batched DVE
rowsum; slices 2-3 as [128,256] exps with ACT accum rowsums). consume runs per
half-quad (reciprocal, in-place vT scale, AV matmuls) lagging DEPTH half-quads
so sequencer waits are pre-satisfied. The next block's x load + y projection
chunks are interleaved between quads.
"""

import sys

import numpy as np

for _p in ("/opt/trn_rl_repo",):
    if _p not in sys.path:
        sys.path.insert(0, _p)

from contextlib import ExitStack

import concourse.bacc as bacc
import concourse.bass as bass
import concourse.mybir as mybir
import concourse.tile as tile
from concourse import bass_utils

F32 = mybir.dt.float32
F32R = mybir.dt.float32r
EXP = mybir.ActivationFunctionType.Exp

C = 64


def _f(ap):
    return ap.bitcast(F32)


def round_f32r(a):
    """Round fp32 array to fp32r (e8m11) bits, round-to-nearest."""
    u = np.ascontiguousarray(a, np.float32).view(np.uint32).copy()
    u = (u + np.uint32(0x800)) & np.uint32(0xFFFFF000)
    return u.view(np.float32)


def build_program(H, W, TW, TH, Gr_np, Wvb_np, r_np):
    nc = bacc.Bacc(
        "TRN2", target_bir_lowering=False, debug=False, enable_asserts=False
    )
    assert H == 256 and W == 256 and TW % 4 == 0 and TH % 4 == 0
    x_d = nc.dram_tensor("x", [C, H, W], F32, kind="ExternalInput")
    out_d = nc.dram_tensor("out", [C, H, W], F32, kind="ExternalOutput")
    Gr_d = nc.inline_tensor(np.ascontiguousarray(round_f32r(Gr_np)), name="Gr")
    Wvb_d = nc.inline_tensor(np.ascontiguousarray(round_f32r(Wvb_np)), name="Wvb")
    r64_d = nc.inline_tensor(r_np.astype(np.float32).reshape(C, 1), name="r64")
    ones_d = nc.inline_tensor(np.ones((1, 512), np.float32), name="ones1")

    with ExitStack() as ctx:
        tc = ctx.enter_context(tile.TileContext(nc))
        _body(ctx, tc, nc, x_d.ap(), out_d.ap(), Gr_d.ap(), Wvb_d.ap(),
              r64_d.ap(), ones_d.ap(), H, W, TW, TH)
    nc.compile()
    return nc


class _Quad:
    __slots__ = ("vtsg", "eg", "ssumg", "L", "store", "avq")

    def __init__(self, vtsg, eg, ssumg, L, store):
        self.vtsg = vtsg
        self.eg = eg
        self.ssumg = ssumg
        self.L = L
        self.store = store
        self.avq = None


def _body(ctx, tc, nc, x, out, Gr, Wvb, r64, ones, H, W, TW, TH):
    consts = ctx.enter_context(tc.tile_pool(name="consts", bufs=1))
    blocks = ctx.enter_context(tc.tile_pool(name="blocks", bufs=2))
    work = ctx.enter_context(tc.tile_pool(name="work", bufs=4))
    epool = ctx.enter_context(tc.tile_pool(name="epool", bufs=4))
    spool = ctx.enter_context(tc.tile_pool(name="spool", bufs=4))
    psum_av = ctx.enter_context(tc.tile_pool(name="psum_av", bufs=1, space="PSUM"))
    psum_lp = ctx.enter_context(tc.tile_pool(name="psum_lp", bufs=4, space="PSUM"))
    psum_vt = ctx.enter_context(tc.tile_pool(name="psum_vt", bufs=1, space="PSUM"))
    psum_yp = ctx.enter_context(tc.tile_pool(name="psum_yp", bufs=1, space="PSUM"))
    dram = ctx.enter_context(tc.tile_pool(name="dram", bufs=1, space="DRAM"))

    Gr_sb = consts.tile([C + 1, C], F32R)
    nc.sync.dma_start(out=Gr_sb[:], in_=Gr.bitcast(F32R))
    Wvb_sb = consts.tile([C + 1, C], F32R)
    nc.sync.dma_start(out=Wvb_sb[:], in_=Wvb.bitcast(F32R))
    r_sb = consts.tile([C, 1], F32)
    nc.sync.dma_start(out=r_sb[:], in_=r64)

    hT = dram.tile([C, W, H], F32)  # gamma*h_out as [c, w, j]
    HT_ap = hT[:]

    # ---------------- software pipeline over half-quads ----------------
    pending = []
    DEPTH = 5

    def produce_quad(lhsT_x, rhs_x, lhsT_y, L, store_quad):
        NI = L // 128
        vtpg = psum_vt.tile([128, 4, NI, C], F32, tag="vtpg")
        eg = epool.tile([128, 4, NI, L], F32R, tag="eg")
        ssumg = spool.tile([128, 4 * NI], F32, tag="ssumg")
        for s in range(4):
            for i in range(NI):
                nc.tensor.matmul(
                    vtpg[:, s, i, :], lhsT=lhsT_x(s, i), rhs=Wvb_sb[:],
                    start=True, stop=True,
                )
        vtsg = spool.tile([128, 4 * NI, C], F32R, tag="vtsg")
        nc.vector.tensor_copy(vtsg[:], vtpg[:].rearrange("p a b c -> p (a b) c"))
        for s in range(4):
            lp = psum_lp.tile([128, NI, L], F32, tag="lp")
            for i in range(NI):
                nc.tensor.matmul(
                    lp[:, i, :], lhsT=lhsT_y(s, i), rhs=rhs_x(s, i),
                    start=True, stop=True,
                )
                if s >= 2:
                    nc.scalar.activation(
                        eg[:, s, i, :], lp[:, i, :], EXP,
                        accum_out=ssumg[:, s * NI + i : s * NI + i + 1],
                    )
            if s < 2:
                nc.scalar.activation(
                    eg[:, s, :, :].rearrange("p a b -> p (a b)"),
                    lp[:].rearrange("p a b -> p (a b)"),
                    EXP,
                )
            if s == 1:
                nc.vector.reduce_sum(
                    out=ssumg[:, 0 : 2 * NI],
                    in_=_f(eg[:, 0:2, :, :]).rearrange("p a b c -> p (a b) c"),
                    axis=mybir.AxisListType.X,
                )
                if len(pending) > DEPTH:
                    consume_half()
        q = _Quad(vtsg, eg, ssumg, L, store_quad)
        pending.append((q, 0))
        pending.append((q, 1))

    def consume_half():
        q, h = pending.pop(0)
        NI = q.L // 128
        k = 2 * NI
        recg = work.tile([128, k], F32, tag="recg")
        nc.vector.reciprocal(recg[:], q.ssumg[:, h * k : (h + 1) * k])
        nc.vector.tensor_mul(
            q.vtsg[:, h * k : (h + 1) * k, :],
            q.vtsg[:, h * k : (h + 1) * k, :],
            recg[:].broadcast_to([128, k, C]),
        )
        if h == 0:
            q.avq = psum_av.tile([C, 4, q.L], F32, tag="avq")  # 2 banks
        for s in (2 * h, 2 * h + 1):
            for i in range(NI):
                nc.tensor.matmul(
                    q.avq[:, s, :],
                    lhsT=q.vtsg[:, s * NI + i, :],
                    rhs=q.eg[:, s, i, :],
                    start=(i == 0), stop=(i == NI - 1),
                )
        if h == 1:
            q.store(q.avq)

    def step_pipeline():
        while len(pending) > DEPTH:
            consume_half()

    def flush_pipeline():
        while pending:
            consume_half()

    # ---------------- block prologue: x load + y projection ----------------
    def prologue_start_col(wb):
        xaug = blocks.tile([C + 1, H, TW], F32R, tag="xaug")
        nc.sync.dma_start(out=xaug[0:C, :, :], in_=x[:, :, _ts(wb, TW)].bitcast(F32R))
        ones_src = bass.AP(
            tensor=ones.tensor, offset=ones.offset, ap=[[0, 1], [0, H], [1, TW]]
        ).bitcast(F32R)
        nc.gpsimd.dma_start(out=xaug[C : C + 1, :, :], in_=ones_src)
        ypk = blocks.tile([C, H, TW], F32R, tag="ypk")
        return xaug, ypk

    def prologue_start_row(hb):
        xaug = blocks.tile([C + 1, TH, W], F32R, tag="xaug")
        nc.sync.dma_start(out=xaug[0:C, :, :], in_=x[:, _ts(hb, TH), :].bitcast(F32R))
        ones_src = bass.AP(
            tensor=ones.tensor, offset=ones.offset, ap=[[0, 1], [0, TH], [1, W]]
        ).bitcast(F32R)
        nc.gpsimd.dma_start(out=xaug[C : C + 1, :, :], in_=ones_src)
        hbt = blocks.tile([C, W, TH], F32, tag="hb")
        nc.sync.dma_start(out=hbt[:], in_=HT_ap[:, :, _ts(hb, TH)])
        ypk = blocks.tile([C, TH, W], F32R, tag="ypk")
        return xaug, hbt, ypk

    def y_chunk(xaug, ypk, g):
        """One 512-position y chunk: y = G^T x + r (fp32r out)."""
        xaug_f = xaug[:].rearrange("p a b -> p (a b)")
        ypk_f = ypk[:].rearrange("p a b -> p (a b)")
        yp = psum_yp.tile([C, 512], F32, tag="yp")
        nc.tensor.matmul(
            yp[:], lhsT=Gr_sb[:], rhs=xaug_f[:, _ts(g, 512)],
            start=True, stop=True,
        )
        nc.vector.tensor_copy(ypk_f[:, _ts(g, 512)], yp[:])

    # ================= Pass 1: column attention =================
    NBW = W // TW
    NQ = TW // 4
    NCH = H * TW // 512
    CPQ = max(1, NCH // NQ)  # y chunks to emit per quad
    cur = prologue_start_col(0)
    for g in range(NCH):
        y_chunk(cur[0], cur[1], g)
    for wb in range(NBW):
        xaug, ypk = cur
        ypk3 = ypk[:]
        nxt = prologue_start_col(wb + 1) if wb + 1 < NBW else None
        gq = iter(range(NCH))
        for wq in range(NQ):
            wp = wq * 4
            w0 = wb * TW + wp

            def store_quad_col(avq, w0=w0):
                hq = work.tile([C, 4, H], F32, tag="hq")
                nc.scalar.copy(
                    hq[:].rearrange("p a b -> p (a b)"),
                    avq[:].rearrange("p a b -> p (a b)"),
                )
                nc.sync.dma_start(out=HT_ap[:, w0 : w0 + 4, :], in_=hq[:])

            produce_quad(
                lhsT_x=lambda s, i, xaug=xaug, wp=wp: xaug[0 : C + 1, _ts(i, 128), wp + s],
                rhs_x=lambda s, i, xaug=xaug, wp=wp: xaug[0:C, :, wp + s],
                lhsT_y=lambda s, i, ypk3=ypk3, wp=wp: ypk3[:, _ts(i, 128), wp + s],
                L=H,
                store_quad=store_quad_col,
            )
            if nxt is not None:
                for _ in range(CPQ):
                    for g in gq:
                        y_chunk(nxt[0], nxt[1], g)
                        break
            step_pipeline()
        if nxt is not None:
            for g in gq:
                y_chunk(nxt[0], nxt[1], g)
        cur = nxt
    flush_pipeline()

    # ================= Pass 2: row attention + combine =================
    NBH = H // TH
    NQH = TH // 4
    NCHR = TH * W // 512
    CPQR = max(1, NCHR // NQH)
    cur = prologue_start_row(0)
    for g in range(NCHR):
        y_chunk(cur[0], cur[2], g)
    for hb in range(NBH):
        xaug2, hbt, ypk2 = cur
        ypk23 = ypk2[:]
        nxt = prologue_start_row(hb + 1) if hb + 1 < NBH else None
        gq = iter(range(NCHR))
        for hq4 in range(NQH):
            hp = hq4 * 4
            h0 = hb * TH + hp

            xhq = spool.tile([C, 4, W], F32, tag="xhq")
            for s in range(4):
                nc.gpsimd.tensor_add(
                    xhq[:, s, :],
                    _f(xaug2[0:C, hp + s, :]),
                    hbt[:, :, hp + s],
                )

            def store_quad_row(avq, h0=h0, xhq=xhq):
                oq = work.tile([C, 4, W], F32, tag="oq")
                nc.vector.tensor_add(
                    oq[:].rearrange("p a b -> p (a b)"),
                    avq[:].rearrange("p a b -> p (a b)"),
                    xhq[:].rearrange("p a b -> p (a b)"),
                )
                nc.sync.dma_start(out=out[:, h0 : h0 + 4, :], in_=oq[:])

            produce_quad(
                lhsT_x=lambda s, i, xaug2=xaug2, hp=hp: xaug2[0 : C + 1, hp + s, _ts(i, 128)],
                rhs_x=lambda s, i, xaug2=xaug2, hp=hp: xaug2[0:C, hp + s, :],
                lhsT_y=lambda s, i, ypk23=ypk23, hp=hp: ypk23[:, hp + s, _ts(i, 128)],
                L=W,
                store_quad=store_quad_row,
            )
            if nxt is not None:
                for _ in range(CPQR):
                    for g in gq:
                        y_chunk(nxt[0], nxt[2], g)
                        break
            step_pipeline()
        if nxt is not None:
            for g in gq:
                y_chunk(nxt[0], nxt[2], g)
        cur = nxt
    flush_pipeline()


def _ts(i, n):
    return slice(i * n, (i + 1) * n)


def _host_weights(Wq, bq, Wk, bk, Wv, bv, gamma):
    g = float(np.asarray(gamma).reshape(-1)[0])
    G = (Wq.astype(np.float64).T @ Wk.astype(np.float64)).astype(np.float32)
    r = (bq.astype(np.float64) @ Wk.astype(np.float64)).astype(np.float32)
    Gr = np.concatenate([G, r[None, :]], axis=0)
    WvTg = (g * Wv.astype(np.float64).T).astype(np.float32)
    bvg = (g * bv.astype(np.float64)).astype(np.float32)
    Wvb = np.concatenate([WvTg, bvg[None, :]], axis=0)
    return Gr, Wvb, r


LAST_EXEC_NS = None
LAST_RESULT = None


def kernel(x, Wq, bq, Wk, bk, Wv, bv, gamma, _tw=16, _th=16, _trace=False, _tmpdir=None):
    global LAST_EXEC_NS, LAST_RESULT
    x = np.asarray(x, dtype=np.float32)
    B, Cin, H, W = x.shape
    assert Cin == C
    Gr, Wvb, r = _host_weights(
        np.asarray(Wq, np.float32), np.asarray(bq, np.float32),
        np.asarray(Wk, np.float32), np.asarray(bk, np.float32),
        np.asarray(Wv, np.float32), np.asarray(bv, np.float32),
        np.asarray(gamma, np.float32),
    )
    nc = build_program(H, W, _tw, _th, Gr, Wvb, r)
    xr = round_f32r(x)
    in_maps = [{"x": np.ascontiguousarray(xr[b])} for b in range(B)]
    res = bass_utils.run_bass_kernel_spmd(
        nc, in_maps, core_ids=list(range(B)), trace=_trace, tmpdir=_tmpdir
    )
    LAST_RESULT = res
    LAST_EXEC_NS = res.exec_time_ns
    out = np.stack([res.results[b]["out"] for b in range(B)], axis=0)
    return out.astype(np.float32)


# revision 7
# speedup vs baseline: 1.0615x; 1.0195x over previous
"""CrissCrossAttention TRN2 kernel v4 — fp32r datapath, software-pipelined.

Math: softmax row-constants cancel, so attn = softmax_j(y_i . x_j) with
y = (Wq^T Wk)^T x + Wk^T bq; gamma folded into Wv/bv on the host.

Per core (B=1 image), two passes: column attention writes gamma*h_out to a
DRAM scratch hT[c, w, j] (contiguous stores); row attention computes
gamma*w_out, adds x + h' (GPSIMD) and stores final rows.

Matmuls stream FP32R (e8m11 — fp32 range, 11-bit mantissa, full PE rate at
N>=256; ~2.4e-4 operand rounding). All matmul operands are produced as f32r:
x / G / Wv rounded on the host and DMA'd as f32r; y via DVE bias-add cast;
exp output cast by ACT. Matmul outputs accumulate in full fp32 PSUM.

Pipeline: quads of 4 slices. produce = vT projections (PSUM evacuated at once
to SBUF), QK logits, exps (slices 0-1 as one wide [128,512] exp + batched DVE
rowsum; slices 2-3 as [128,256] exps with ACT accum rowsums). consume runs per
half-quad (reciprocal, in-place vT scale, AV matmuls) lagging DEPTH half-quads
so sequencer waits are pre-satisfied. The next block's x load + y projection
chunks are interleaved between quads.
"""

import sys

import numpy as np

for _p in ("/opt/trn_rl_repo",):
    if _p not in sys.path:
        sys.path.insert(0, _p)

from contextlib import ExitStack

import concourse.bacc as bacc
import concourse.bass as bass
import concourse.mybir as mybir
import concourse.tile as tile
from concourse import bass_utils

F32 = mybir.dt.float32
F32R = mybir.dt.float32r
EXP = mybir.ActivationFunctionType.Exp

C = 64


def _f(ap):
    return ap.bitcast(F32)


def round_f32r(a):
    """Round fp32 array to fp32r (e8m11) bits, round-to-nearest."""
    u = np.ascontiguousarray(a, np.float32).view(np.uint32).copy()
    u = (u + np.uint32(0x800)) & np.uint32(0xFFFFF000)
    return u.view(np.float32)


def build_program(H, W, TW, TH, Gr_np, Wvb_np, r_np):
    nc = bacc.Bacc(
        "TRN2", target_bir_lowering=False, debug=False, enable_asserts=False
    )
    assert H == 256 and W == 256 and TW % 4 == 0 and TH % 4 == 0
    x_d = nc.dram_tensor("x", [C, H, W], F32, kind="ExternalInput")
    out_d = nc.dram_tensor("out", [C, H, W], F32, kind="ExternalOutput")
    Gr_d = nc.inline_tensor(np.ascontiguousarray(round_f32r(Gr_np)), name="Gr")
    Wvb_d = nc.inline_tensor(np.ascontiguousarray(round_f32r(Wvb_np)), name="Wvb")
    r64_d = nc.inline_tensor(r_np.astype(np.float32).reshape(C, 1), name="r64")
    ones_d = nc.inline_tensor(np.ones((1, 512), np.float32), name="ones1")

    with ExitStack() as ctx:
        tc = ctx.enter_context(tile.TileContext(nc))
        _body(ctx, tc, nc, x_d.ap(), out_d.ap(), Gr_d.ap(), Wvb_d.ap(),
              r64_d.ap(), ones_d.ap(), H, W, TW, TH)
    nc.compile()
    return nc


class _Quad:
    __slots__ = ("vtsg", "eg", "ssumg", "L", "store", "avq")

    def __init__(self, vtsg, eg, ssumg, L, store):
        self.vtsg = vtsg
        self.eg = eg
        self.ssumg = ssumg
        self.L = L
        self.store = store
        self.avq = None


def _body(ctx, tc, nc, x, out, Gr, Wvb, r64, ones, H, W, TW, TH):
    consts = ctx.enter_context(tc.tile_pool(name="consts", bufs=1))
    blocks = ctx.enter_context(tc.tile_pool(name="blocks", bufs=2))
    work = ctx.enter_context(tc.tile_pool(name="work", bufs=4))
    epool = ctx.enter_context(tc.tile_pool(name="epool", bufs=4))
    spool = ctx.enter_context(tc.tile_pool(name="spool", bufs=4))
    psum_av = ctx.enter_context(tc.tile_pool(name="psum_av", bufs=1, space="PSUM"))
    psum_lp = ctx.enter_context(tc.tile_pool(name="psum_lp", bufs=4, space="PSUM"))
    psum_vt = ctx.enter_context(tc.tile_pool(name="psum_vt", bufs=1, space="PSUM"))
    psum_yp = ctx.enter_context(tc.tile_pool(name="psum_yp", bufs=1, space="PSUM"))
    dram = ctx.enter_context(tc.tile_pool(name="dram", bufs=1, space="DRAM"))

    Gr_sb = consts.tile([C + 1, C], F32R)
    nc.sync.dma_start(out=Gr_sb[:], in_=Gr.bitcast(F32R))
    Wvb_sb = consts.tile([C + 1, C], F32R)
    nc.sync.dma_start(out=Wvb_sb[:], in_=Wvb.bitcast(F32R))
    r_sb = consts.tile([C, 1], F32)
    nc.sync.dma_start(out=r_sb[:], in_=r64)

    hT = dram.tile([C, W, H], F32)  # gamma*h_out as [c, w, j]
    HT_ap = hT[:]

    # ---------------- software pipeline over half-quads ----------------
    pending = []
    DEPTH = 5

    def produce_quad(lhsT_x, rhs_x, lhsT_y, L, store_quad):
        NI = L // 128
        vtpg = psum_vt.tile([128, 4, NI, C], F32, tag="vtpg")
        eg = epool.tile([128, 4, NI, L], F32R, tag="eg")
        ssumg = spool.tile([128, 4 * NI], F32, tag="ssumg")
        for s in range(4):
            for i in range(NI):
                nc.tensor.matmul(
                    vtpg[:, s, i, :], lhsT=lhsT_x(s, i), rhs=Wvb_sb[:],
                    start=True, stop=True,
                )
        vtsg = spool.tile([128, 4 * NI, C], F32R, tag="vtsg")
        nc.vector.tensor_copy(vtsg[:], vtpg[:].rearrange("p a b c -> p (a b) c"))
        for s in range(4):
            lp = psum_lp.tile([128, NI, L], F32, tag="lp")
            for i in range(NI):
                nc.tensor.matmul(
                    lp[:, i, :], lhsT=lhsT_y(s, i), rhs=rhs_x(s, i),
                    start=True, stop=True,
                )
                if s >= 2:
                    nc.scalar.activation(
                        eg[:, s, i, :], lp[:, i, :], EXP,
                        accum_out=ssumg[:, s * NI + i : s * NI + i + 1],
                    )
            if s < 2:
                nc.scalar.activation(
                    eg[:, s, :, :].rearrange("p a b -> p (a b)"),
                    lp[:].rearrange("p a b -> p (a b)"),
                    EXP,
                )
            if s == 1:
                nc.vector.reduce_sum(
                    out=ssumg[:, 0 : 2 * NI],
                    in_=_f(eg[:, 0:2, :, :]).rearrange("p a b c -> p (a b) c"),
                    axis=mybir.AxisListType.X,
                )
                if len(pending) > DEPTH:
                    consume_half()
        q = _Quad(vtsg, eg, ssumg, L, store_quad)
        pending.append((q, 0))
        pending.append((q, 1))

    def consume_half():
        q, h = pending.pop(0)
        NI = q.L // 128
        k = 2 * NI
        recg = work.tile([128, k], F32, tag="recg")
        nc.vector.reciprocal(recg[:], q.ssumg[:, h * k : (h + 1) * k])
        nc.vector.tensor_mul(
            q.vtsg[:, h * k : (h + 1) * k, :],
            q.vtsg[:, h * k : (h + 1) * k, :],
            recg[:].broadcast_to([128, k, C]),
        )
        if h == 0:
            q.avq = psum_av.tile([C, 4, q.L], F32, tag="avq")  # 2 banks
        for s in (2 * h, 2 * h + 1):
            for i in range(NI):
                nc.tensor.matmul(
                    q.avq[:, s, :],
                    lhsT=q.vtsg[:, s * NI + i, :],
                    rhs=q.eg[:, s, i, :],
                    start=(i == 0), stop=(i == NI - 1),
                )
        if h == 1:
            q.store(q.avq)

    def step_pipeline():
        while len(pending) > DEPTH:
            consume_half()

    def flush_pipeline():
        while pending:
            consume_half()

    # ---------------- block prologue: x load + y projection ----------------
    def prologue_start_col(wb):
        xaug = blocks.tile([C + 1, H, TW], F32R, tag="xaug")
        nc.sync.dma_start(out=xaug[0:C, :, :], in_=x[:, :, _ts(wb, TW)].bitcast(F32R))
        ones_src = bass.AP(
            tensor=ones.tensor, offset=ones.offset, ap=[[0, 1], [0, H], [1, TW]]
        ).bitcast(F32R)
        nc.gpsimd.dma_start(out=xaug[C : C + 1, :, :], in_=ones_src)
        ypk = blocks.tile([C, H, TW], F32R, tag="ypk")
        return xaug, ypk

    def prologue_start_row(hb):
        xaug = blocks.tile([C + 1, TH, W], F32R, tag="xaug")
        nc.sync.dma_start(out=xaug[0:C, :, :], in_=x[:, _ts(hb, TH), :].bitcast(F32R))
        ones_src = bass.AP(
            tensor=ones.tensor, offset=ones.offset, ap=[[0, 1], [0, TH], [1, W]]
        ).bitcast(F32R)
        nc.gpsimd.dma_start(out=xaug[C : C + 1, :, :], in_=ones_src)
        hbt = blocks.tile([C, W, TH], F32, tag="hb")
        nc.sync.dma_start(out=hbt[:], in_=HT_ap[:, :, _ts(hb, TH)])
        ypk = blocks.tile([C, TH, W], F32R, tag="ypk")
        return xaug, hbt, ypk

    def y_chunk(xaug, ypk, g):
        """One 512-position y chunk: y = G^T x + r (fp32r out)."""
        xaug_f = xaug[:].rearrange("p a b -> p (a b)")
        ypk_f = ypk[:].rearrange("p a b -> p (a b)")
        yp = psum_yp.tile([C, 512], F32, tag="yp")
        nc.tensor.matmul(
            yp[:], lhsT=Gr_sb[:], rhs=xaug_f[:, _ts(g, 512)],
            start=True, stop=True,
        )
        nc.vector.tensor_copy(ypk_f[:, _ts(g, 512)], yp[:])

    # ================= Pass 1: column attention =================
    NBW = W // TW
    NQ = TW // 4
    NCH = H * TW // 512
    CPQ = max(1, NCH // NQ)  # y chunks to emit per quad
    cur = prologue_start_col(0)
    for g in range(NCH):
        y_chunk(cur[0], cur[1], g)
    for wb in range(NBW):
        xaug, ypk = cur
        ypk3 = ypk[:]
        nxt = prologue_start_col(wb + 1) if wb + 1 < NBW else None
        gq = iter(range(NCH))
        for wq in range(NQ):
            wp = wq * 4
            w0 = wb * TW + wp

            def store_quad_col(avq, w0=w0):
                hq = work.tile([C, 4, H], F32, tag="hq")
                nc.scalar.copy(
                    hq[:].rearrange("p a b -> p (a b)"),
                    avq[:].rearrange("p a b -> p (a b)"),
                )
                nc.gpsimd.dma_start(out=HT_ap[:, w0 : w0 + 4, :], in_=hq[:])

            produce_quad(
                lhsT_x=lambda s, i, xaug=xaug, wp=wp: xaug[0 : C + 1, _ts(i, 128), wp + s],
                rhs_x=lambda s, i, xaug=xaug, wp=wp: xaug[0:C, :, wp + s],
                lhsT_y=lambda s, i, ypk3=ypk3, wp=wp: ypk3[:, _ts(i, 128), wp + s],
                L=H,
                store_quad=store_quad_col,
            )
            if nxt is not None:
                for _ in range(CPQ):
                    for g in gq:
                        y_chunk(nxt[0], nxt[1], g)
                        break
            step_pipeline()
        if nxt is not None:
            for g in gq:
                y_chunk(nxt[0], nxt[1], g)
        cur = nxt
    flush_pipeline()

    # ================= Pass 2: row attention + combine =================
    NBH = H // TH
    NQH = TH // 4
    NCHR = TH * W // 512
    CPQR = max(1, NCHR // NQH)
    cur = prologue_start_row(0)
    for g in range(NCHR):
        y_chunk(cur[0], cur[2], g)
    for hb in range(NBH):
        xaug2, hbt, ypk2 = cur
        ypk23 = ypk2[:]
        nxt = prologue_start_row(hb + 1) if hb + 1 < NBH else None
        gq = iter(range(NCHR))
        for hq4 in range(NQH):
            hp = hq4 * 4
            h0 = hb * TH + hp

            xhq = spool.tile([C, 4, W], F32, tag="xhq")
            for s in range(4):
                nc.gpsimd.tensor_add(
                    xhq[:, s, :],
                    _f(xaug2[0:C, hp + s, :]),
                    hbt[:, :, hp + s],
                )

            def store_quad_row(avq, h0=h0, xhq=xhq):
                oq = work.tile([C, 4, W], F32, tag="oq")
                nc.vector.tensor_add(
                    oq[:].rearrange("p a b -> p (a b)"),
                    avq[:].rearrange("p a b -> p (a b)"),
                    xhq[:].rearrange("p a b -> p (a b)"),
                )
                nc.gpsimd.dma_start(out=out[:, h0 : h0 + 4, :], in_=oq[:])

            produce_quad(
                lhsT_x=lambda s, i, xaug2=xaug2, hp=hp: xaug2[0 : C + 1, hp + s, _ts(i, 128)],
                rhs_x=lambda s, i, xaug2=xaug2, hp=hp: xaug2[0:C, hp + s, :],
                lhsT_y=lambda s, i, ypk23=ypk23, hp=hp: ypk23[:, hp + s, _ts(i, 128)],
                L=W,
                store_quad=store_quad_row,
            )
            if nxt is not None:
                for _ in range(CPQR):
                    for g in gq:
                        y_chunk(nxt[0], nxt[2], g)
                        break
            step_pipeline()
        if nxt is not None:
            for g in gq:
                y_chunk(nxt[0], nxt[2], g)
        cur = nxt
    flush_pipeline()


def _ts(i, n):
    return slice(i * n, (i + 1) * n)


def _host_weights(Wq, bq, Wk, bk, Wv, bv, gamma):
    g = float(np.asarray(gamma).reshape(-1)[0])
    G = (Wq.astype(np.float64).T @ Wk.astype(np.float64)).astype(np.float32)
    r = (bq.astype(np.float64) @ Wk.astype(np.float64)).astype(np.float32)
    Gr = np.concatenate([G, r[None, :]], axis=0)
    WvTg = (g * Wv.astype(np.float64).T).astype(np.float32)
    bvg = (g * bv.astype(np.float64)).astype(np.float32)
    Wvb = np.concatenate([WvTg, bvg[None, :]], axis=0)
    return Gr, Wvb, r


LAST_EXEC_NS = None
LAST_RESULT = None


def kernel(x, Wq, bq, Wk, bk, Wv, bv, gamma, _tw=16, _th=16, _trace=False, _tmpdir=None):
    global LAST_EXEC_NS, LAST_RESULT
    x = np.asarray(x, dtype=np.float32)
    B, Cin, H, W = x.shape
    assert Cin == C
    Gr, Wvb, r = _host_weights(
        np.asarray(Wq, np.float32), np.asarray(bq, np.float32),
        np.asarray(Wk, np.float32), np.asarray(bk, np.float32),
        np.asarray(Wv, np.float32), np.asarray(bv, np.float32),
        np.asarray(gamma, np.float32),
    )
    nc = build_program(H, W, _tw, _th, Gr, Wvb, r)
    xr = round_f32r(x)
    in_maps = [{"x": np.ascontiguousarray(xr[b])} for b in range(B)]
    res = bass_utils.run_bass_kernel_spmd(
        nc, in_maps, core_ids=list(range(B)), trace=_trace, tmpdir=_tmpdir
    )
    LAST_RESULT = res
    LAST_EXEC_NS = res.exec_time_ns
    out = np.stack([res.results[b]["out"] for b in range(B)], axis=0)
    return out.astype(np.float32)
